# revision 13
# baseline (speedup 1.0000x reference)
"""Expert-parallel MoE feed-forward (top-2 routing) on 8 TRN2 NeuronCores.

Strategy: one expert per core (E == n_cores == 8). Token routing is part of
input sharding: host gathers each expert's assigned token activations
(transposed, bf16) and feeds core e only its tokens plus its expert's three
weight matrices. Each core runs a dense FFN
    out = (silu(x @ Wg^T) * (x @ Wu^T)) @ Wd^T
over its token batch in bf16 (fp32 PSUM accumulation), entirely from SBUF.
Host scatters per-core outputs back into the (T, A, D) result.
"""

import math
import sys
import types

import numpy as np
import ml_dtypes

T, D, H, E, A = 4096, 1024, 2048, 8, 2
N_CORES = 8
BF16 = ml_dtypes.bfloat16

# Filled by kernel() with the BassKernelResults of the last device run so an
# external harness (test.py) can read exec_time_ns when tracing is on.
LAST_RESULT = None

_SHIMS_DONE = False


def _install_shims():
    """Environment fixes for running Bass/Tile SPMD kernels under axon."""
    global _SHIMS_DONE
    if _SHIMS_DONE:
        return
    _SHIMS_DONE = True

    # 1. NTFF profile hook (lets trace=True / BASS_TRACE=1 report exec_time_ns).
    if "antenv.axon_hooks" not in sys.modules:
        try:
            import antenv.axon_hooks  # noqa: F401  (real module present)
        except ImportError:
            _hook = None
            try:
                import trn_agent_boot.trn_boot as tb

                _hook = tb._ntff_profile_via_ctypes("/opt/axon/libaxon_pjrt.so")
            except Exception:
                _hook = None
            mod = types.ModuleType("antenv.axon_hooks")
            mod.get_axon_ntff_profile_hook = lambda: _hook
            sys.modules["antenv.axon_hooks"] = mod

    # 2. No artifact upload from a zero-egress container.
    from concourse import bass_utils

    bass_utils.upload_artifacts = lambda tmpdir: f"local:{tmpdir}"

    # 3. This walrus build allows only one sync-wait command on a CTRL
    # (Drain) instruction; split the tile-exit drain's waits onto nops.
    import concourse.tile as tile
    from concourse import mybir
    from concourse.vector_clock import ScopedClock

    if getattr(tile.TileContext._drain_and_barrier, "_is_patched", False):
        return

    def _patched_drain_and_barrier(self, tick_clock, wait_clock):
        nc = self.nc
        drain_inst = nc.sync.drain()
        wait_clock.add_sem_waits(
            drain_inst.ins, ScopedClock({None: tick_clock.global_clock})
        )
        ow = drain_inst.ins.sync_info.on_wait if drain_inst.ins.sync_info else None
        maxw = 1
        if ow and len(ow) > maxw:
            extra = list(ow[maxw:])
            del ow[maxw:]
            for i in range(0, len(extra), maxw):
                nop = nc.sync.nop(hint="drain_split", nofuse=True)
                if nop.ins.sync_info is None:
                    nop.ins.sync_info = mybir.SyncInfo(on_wait=[], on_update=[])
                for w in extra[i : i + maxw]:
                    nop.ins.sync_info.on_wait.append(w)
        nc.all_engine_barrier()
        assert self.sems is not None
        popped = nc._tile_sem_poison_stack.pop()
        assert popped is self._sem_poison
        nc.clear_and_free_semaphores(list(self.sems.allocated().values()))
        nc.all_engine_barrier()

    _patched_drain_and_barrier._is_patched = True
    tile.TileContext._drain_and_barrier = _patched_drain_and_barrier


def _split_multi_waits(nc):
    """This walrus build allows one sync-wait command per instruction.

    Tile's sem assignment can attach several; move the extras onto nofuse
    NoOps inserted just before the instruction on the same engine (engines
    execute a block's instructions in order, so semantics are unchanged).
    """
    import bass_rust
    from concourse import mybir

    ctr = 0
    for f in nc.m.functions:
        for bb in f.blocks:
            new = []
            changed = False
            for inst in bb.instructions:
                si = inst.sync_info
                ow = si.on_wait if si else None
                if ow is not None and len(ow) > 1:
                    extra = list(ow[:-1])
                    del ow[:-1]
                    for w in extra:
                        ctr += 1
                        nop = bass_rust.InstNoOp()
                        nop.name = f"I-wsplit-{ctr}"
                        nop.engine = inst.engine
                        nop.sync_info = mybir.SyncInfo(on_wait=[w], on_update=[])
                        nop.bass_nofuse = True
                        new.append(nop)
                    changed = True
                new.append(inst)
            if changed:
                bb.instructions = new


def _chunk_sizes(cap):
    """Split cap token columns into chunks of <=512 (PSUM bank limit).

    The first chunk is as large as possible: it runs while the weights are
    still streaming in from HBM, and a wider chunk does more PE work per
    weight byte (lower demand bandwidth during the ramp). The remaining
    columns are split near-equally so no chunk is tiny (per-matmul overhead
    is paid per chunk; a tiny chunk is LDWEIGHTS-bound).
    """
    if cap <= 512:
        return [cap]
    first = 512
    rest = cap - first
    n = max(1, math.ceil(rest / 512))
    base = rest // n
    rem = rest - base * n
    return [first] + [base + (1 if i < rem else 0) for i in range(n)]


_NC_CACHE = {}


def _build_nc(cap):
    if cap in _NC_CACHE:
        return _NC_CACHE[cap]
    import concourse.bass as bass
    import concourse.tile as tile
    from concourse import mybir

    f32 = mybir.dt.float32
    bf16 = mybir.dt.bfloat16
    KD = D // 128  # 8  k-tiles over the model dim
    KH = H // 128  # 16 k-tiles over the hidden dim
    chunks = _chunk_sizes(cap)
    cmax = max(chunks)

    nc = bass.Bass()
    xT = nc.dram_tensor("xT", [D, cap], bf16, kind="ExternalInput")
    wgT = nc.dram_tensor("wgT", [D, H], bf16, kind="ExternalInput")
    wuT = nc.dram_tensor("wuT", [D, H], bf16, kind="ExternalInput")
    wdT = nc.dram_tensor("wdT", [H, D], bf16, kind="ExternalInput")
    out = nc.dram_tensor("out", [D, cap], bf16, kind="ExternalOutput")

    WCH = 512  # weight DMA column-chunk
    c_offs = []
    c0 = 0
    for cn in chunks:
        c_offs.append((c0, cn))
        c0 += cn

    with tile.TileContext(nc) as tc:
        with (
            tc.tile_pool(name="wpool", bufs=1) as wpool,
            tc.tile_pool(name="hpool", bufs=2) as hpool,
            tc.tile_pool(name="opool", bufs=4) as opool,
            tc.tile_pool(name="psum", bufs=2, space="PSUM") as psum,
        ):
            # Input DMAs ordered by first use, chunked wide enough that each
            # transfer keeps full-row descriptors (>=1 KB rows).
            x_sb = [
                wpool.tile([128, cap], bf16, tag=f"x{ki}", name=f"x_sb{ki}")
                for ki in range(KD)
            ]
            wg_sb = [
                wpool.tile([128, H], bf16, tag=f"wg{ki}", name=f"wg_sb{ki}")
                for ki in range(KD)
            ]
            wu_sb = [
                wpool.tile([128, H], bf16, tag=f"wu{ki}", name=f"wu_sb{ki}")
                for ki in range(KD)
            ]
            wd_sb = [
                wpool.tile([128, D], bf16, tag=f"wd{hk}", name=f"wd_sb{hk}")
                for hk in range(KH)
            ]
            # The PE clock-gate (HAM) needs ~3.4 us of sustained tensor-engine
            # activity before it releases the 1.2 GHz cold throttle. Real
            # matmuls can't start until the first weight/x DMAs land (~9 us:
            # ~8 us framework preamble + transfer), so burn the DMA wait on
            # dummy self-loading matmuls over an SBUF scratch tile. By the
            # time data arrives the PE is at full clock instead of spending
            # the first ~3.4 us of real matmuls at half speed.
            scr = wpool.tile([128, 128], bf16, tag="scr", name="scr")
            nc.any.memset(scr[:], 0.0)
            wm = psum.tile([128, 512], f32, tag="po", name="po")
            for _ in range(16):
                nc.tensor.matmul(
                    wm[:, :64], scr[:, :], scr[:, :64], start=True, stop=True
                )
            # Each dma_start costs ~0.6 us of SERIAL issue time on its issuing
            # engine's sequencer, so spread the input loads over three issuing
            # engines (SP, GpSimd, ACT), ordered by consumption deadline.
            cn0 = chunks[0]

            def dma_wg(eng, wc):
                for ki in range(KD):
                    eng.dma_start(
                        wg_sb[ki][:, wc : wc + WCH],
                        wgT[128 * ki : 128 * (ki + 1), wc : wc + WCH],
                    )

            # HBM crunch window is [~9, ~24] us: the gate phase of chunk 0
            # consumes x-chunk0 (1 MB) + all of wg (4 MB) nearly as fast as
            # 358 GB/s can deliver. ONLY those transfers may run then — wu /
            # x-chunk1 / wd are issued later (issue time gates transfer
            # start) so they don't steal bandwidth and stall the PE ramp.
            # GpSimd: gate weight chunk 0 (chunk 1 goes to ACT).
            dma_wg(nc.gpsimd, 0)
            # ACT: gate weight chunk 1, issued before any silu work exists.
            dma_wg(nc.scalar, WCH)
            # SP: x for token-chunk 0 (needed by the very first matmul).
            for ki in range(KD):
                nc.sync.dma_start(
                    x_sb[ki][:, :cn0], xT[128 * ki : 128 * (ki + 1), :cn0]
                )
            dma_wg(nc.gpsimd, 2 * WCH)
            dma_wg(nc.gpsimd, 3 * WCH)
            # GpSimd: rest of x (needed by gate(chunk1), ~65 us in) — queued
            # behind wg so its transfers start only once wg is done.
            if cn0 < cap:
                for ki in range(KD):
                    nc.gpsimd.dma_start(
                        x_sb[ki][:, cn0:], xT[128 * ki : 128 * (ki + 1), cn0:]
                    )
            # GpSimd: down weights whole-tile (needed last).
            for hk in range(KH):
                nc.gpsimd.dma_start(wd_sb[hk][:], wdT[128 * hk : 128 * (hk + 1), :])

            def dma_wu(half):
                # Issued from ACT *between* gate-phase writer groups (see
                # gate_up): transfers begin ~22/~30 us, after the wg crunch,
                # and complete before the up phase needs them at ~38 us.
                wc = half * 2 * WCH
                for ki in range(KD):
                    nc.scalar.dma_start(
                        wu_sb[ki][:, wc : wc + 2 * WCH],
                        wuT[128 * ki : 128 * (ki + 1), wc : wc + 2 * WCH],
                    )

            def gate_up(c0, cn, emit_wu=False):
                # Phase 1: all gate matmuls; silu lands bf16 directly in h.
                # Phase 2: all up matmuls; h *= pu in place on the DVE.
                # Phasing delays the first need for wu by a whole gate phase.
                # Within a phase, k is the OUTER loop over groups of 6 h-tiles
                # accumulating in 6 PSUM banks: weight consumption order then
                # matches the k-major DMA arrival order, so the PE never
                # outruns the transfer frontier during the startup ramp.
                h_sb = hpool.tile([128, KH * cmax], bf16, tag="h", name="h_sb")
                csl = slice(c0, c0 + cn)

                def phase(w_sb, writer, hooks=()):
                    # First group is 4 h-tiles: it only needs weight columns
                    # 0:512 (one DMA chunk) plus x chunk 0, so the PE can
                    # start ~2 us sooner and tracks the HBM arrival rate
                    # during the ramp instead of stalling on chunk 1.
                    groups = [(0, 4), (4, 10), (10, 16)]
                    for gi, (glo, ghi) in enumerate(groups):
                        his = range(glo, ghi)
                        pp = [
                            psum.tile(
                                [128, 512], f32, tag=f"pp{j}", bufs=1, name=f"pp{j}"
                            )
                            for j in range(len(his))
                        ]
                        for ki in range(KD):
                            for j, hi in enumerate(his):
                                nc.tensor.matmul(
                                    pp[j][:, :cn],
                                    w_sb[ki][:, 128 * hi : 128 * (hi + 1)],
                                    x_sb[ki][:, csl],
                                    start=(ki == 0),
                                    stop=(ki == KD - 1),
                                )
                        for j, hi in enumerate(his):
                            writer(hi, pp[j])
                        if gi < len(hooks):
                            hooks[gi]()

                def gate_writer(hi, pp):
                    nc.scalar.activation(
                        h_sb[:, cmax * hi : cmax * hi + cn],
                        pp[:, :cn],
                        mybir.ActivationFunctionType.Silu,
                    )

                def up_writer(hi, pp):
                    hslc = slice(cmax * hi, cmax * hi + cn)
                    nc.vector.tensor_mul(h_sb[:, hslc], h_sb[:, hslc], pp[:, :cn])

                hooks = (
                    (lambda: dma_wu(0), lambda: dma_wu(1)) if emit_wu else ()
                )
                phase(wg_sb, gate_writer, hooks)
                phase(wu_sb, up_writer)
                return h_sb

            def down(h_sb, c0, cn):
                # Rotate PSUM banks through all 8 (the 6 gate/up banks are
                # idle by now): with only 2 "po" banks, each d-tile's first
                # matmul hit a WAR stall (~0.5-0.8 us x 16) waiting for the
                # previous d-tile's PSUM->SBUF copy. An 8-deep rotation gives
                # the copy several d-tile spans to finish.
                last_chunk = (c0, cn) == c_offs[-1]
                # GpSimd's (and likely Scalar's) tile-exit drain POLLS its
                # hardware DMA queues and overshoots an in-flight transfer by
                # ~3 us; Sync's drain sem-waits exactly. So gpsimd/scalar
                # only issue stores for EARLY d-tiles (their queues are empty
                # by kernel end) and everything near the tail goes via Sync.
                out_eng = [
                    nc.gpsimd, nc.gpsimd, nc.scalar, nc.scalar,
                    nc.sync, nc.sync, nc.sync, nc.sync,
                ]
                for di in range(KD):
                    dsl = slice(128 * di, 128 * (di + 1))
                    if di < 6:
                        po = psum.tile(
                            [128, 512], f32, tag=f"pp{di}", bufs=1, name=f"pp{di}"
                        )
                    else:
                        po = psum.tile([128, 512], f32, tag="po", name="po")
                    hb = cmax  # h_sb column stride per k-tile
                    if last_chunk and di == KD - 1:
                        # Final tile is the exec-time tail: accumulate it in
                        # two column halves so the first half's PSUM drain and
                        # store run under the second half's matmuls; only the
                        # second (quarter-size) copy+store is exposed.
                        h1 = cn // 2
                        o = opool.tile([128, 512], bf16, tag="o", name="o")
                        # second half gets its own PSUM bank: tile tracks the
                        # psum tile coarsely, so reusing one bank made half
                        # B's first matmul wait ~0.9 us for half A's copy.
                        po_b = psum.tile([128, 512], f32, tag="po", name="po")
                        for lo, hi, cp, pb in (
                            (0, h1, nc.vector, po),
                            (h1, cn, nc.scalar, po_b),
                        ):
                            for hk in range(KH):
                                nc.tensor.matmul(
                                    pb[:, lo:hi],
                                    wd_sb[hk][:, dsl],
                                    h_sb[:, hb * hk + lo : hb * hk + hi],
                                    start=(hk == 0),
                                    stop=(hk == KH - 1),
                                )
                            if cp is nc.vector:
                                cp.tensor_copy(o[:, lo:hi], pb[:, lo:hi])
                            else:
                                cp.activation(
                                    o[:, lo:hi],
                                    pb[:, lo:hi],
                                    mybir.ActivationFunctionType.Copy,
                                )
                            nc.sync.dma_start(
                                out[dsl, c0 + lo : c0 + hi], o[:, lo:hi]
                            )
                    else:
                        for hk in range(KH):
                            nc.tensor.matmul(
                                po[:, :cn],
                                wd_sb[hk][:, dsl],
                                h_sb[:, hb * hk : hb * hk + cn],
                                start=(hk == 0),
                                stop=(hk == KH - 1),
                            )
                        o = opool.tile([128, 512], bf16, tag="o", name="o")
                        nc.vector.tensor_copy(o[:, :cn], po[:, :cn])
                        # spread issue cost (~0.7 us each) over idle engines
                        out_eng[di].dma_start(out[dsl, c0 : c0 + cn], o[:, :cn])

            # Software-pipelined emission: down(c) goes after gate_up(c+1) so
            # the PE can run chunk c+1's gate matmuls while the DVE finishes
            # chunk c's h tiles (h is double-buffered).
            prev = None
            for idx, (c0i, cni) in enumerate(c_offs):
                h_sb = gate_up(c0i, cni, emit_wu=(idx == 0))
                if prev is not None:
                    down(*prev)
                prev = (h_sb, c0i, cni)
            down(*prev)
    _split_multi_waits(nc)
    _NC_CACHE[cap] = nc
    return nc


def kernel(x, expert_indices, w_gate, w_up, w_down):
    global LAST_RESULT
    _install_shims()
    from concourse import bass_utils

    x = np.asarray(x)
    ei = np.asarray(expert_indices).astype(np.int64)
    w_gate = np.asarray(w_gate)
    w_up = np.asarray(w_up)
    w_down = np.asarray(w_down)

    flat = ei.reshape(-1)  # pair p = t*A + a  ->  expert id
    # Dedup: a (token, slot) pair whose expert already appears in an earlier
    # slot of the same token produces an identical output row — compute the
    # first occurrence only and copy the result to the duplicates afterward.
    keep = np.ones(T * A, dtype=bool)
    for a in range(1, A):
        dup_any = np.zeros(T, dtype=bool)
        for b in range(a):
            dup_any |= ei[:, a] == ei[:, b]
        keep[a::A] = ~dup_any[: T]
    kept = np.nonzero(keep)[0]
    flat_kept = flat[kept]
    counts = np.bincount(flat_kept, minlength=E)
    order = np.argsort(flat_kept, kind="stable")
    starts = np.zeros(E + 1, dtype=np.int64)
    np.cumsum(counts, out=starts[1:])
    cap = int(counts.max())
    cap = max(cap, 128)

    idx_per_core = []
    in_maps = []
    for e in range(E):
        idx = kept[order[starts[e] : starts[e + 1]]]  # original pair ids
        idx_per_core.append(idx)
        tok = idx // A
        xeT = np.zeros((D, cap), dtype=BF16)
        xeT[:, : len(idx)] = x[tok].T.astype(BF16)
        in_maps.append(
            {
                "xT": xeT,
                "wgT": np.ascontiguousarray(w_gate[e].T).astype(BF16),
                "wuT": np.ascontiguousarray(w_up[e].T).astype(BF16),
                "wdT": np.ascontiguousarray(w_down[e].T).astype(BF16),
            }
        )

    nc = _build_nc(cap)
    res = bass_utils.run_bass_kernel_spmd(nc, in_maps, core_ids=list(range(N_CORES)))
    LAST_RESULT = res

    out = np.zeros((T * A, D), dtype=np.float32)
    for e in range(E):
        idx = idx_per_core[e]
        oT = np.asarray(res.results[e]["out"])  # [D, cap] bf16
        out[idx] = oT[:, : len(idx)].T.astype(np.float32)
    out = out.reshape(T, A, D)
    for a in range(1, A):  # fill duplicate slots from their first occurrence
        for b in range(a):
            m = ei[:, a] == ei[:, b]
            if b > 0:
                for c in range(b):
                    m &= ei[:, b] != ei[:, c]  # b is itself the first occurrence
            out[m, a] = out[m, b]
    return out



# revision 14
# speedup vs baseline: 1.0179x; 1.0179x over previous
"""Expert-parallel MoE feed-forward (top-2 routing) on 8 TRN2 NeuronCores.

Strategy: one expert per core (E == n_cores == 8). Token routing is part of
input sharding: host gathers each expert's assigned token activations
(transposed, bf16) and feeds core e only its tokens plus its expert's three
weight matrices. Each core runs a dense FFN
    out = (silu(x @ Wg^T) * (x @ Wu^T)) @ Wd^T
over its token batch in bf16 (fp32 PSUM accumulation), entirely from SBUF.
Host scatters per-core outputs back into the (T, A, D) result.
"""

import math
import sys
import types

import numpy as np
import ml_dtypes

T, D, H, E, A = 4096, 1024, 2048, 8, 2
N_CORES = 8
BF16 = ml_dtypes.bfloat16

# Filled by kernel() with the BassKernelResults of the last device run so an
# external harness (test.py) can read exec_time_ns when tracing is on.
LAST_RESULT = None

_SHIMS_DONE = False


def _install_shims():
    """Environment fixes for running Bass/Tile SPMD kernels under axon."""
    global _SHIMS_DONE
    if _SHIMS_DONE:
        return
    _SHIMS_DONE = True

    # 1. NTFF profile hook (lets trace=True / BASS_TRACE=1 report exec_time_ns).
    if "antenv.axon_hooks" not in sys.modules:
        try:
            import antenv.axon_hooks  # noqa: F401  (real module present)
        except ImportError:
            _hook = None
            try:
                import trn_agent_boot.trn_boot as tb

                _hook = tb._ntff_profile_via_ctypes("/opt/axon/libaxon_pjrt.so")
            except Exception:
                _hook = None
            mod = types.ModuleType("antenv.axon_hooks")
            mod.get_axon_ntff_profile_hook = lambda: _hook
            sys.modules["antenv.axon_hooks"] = mod

    # 2. No artifact upload from a zero-egress container.
    from concourse import bass_utils

    bass_utils.upload_artifacts = lambda tmpdir: f"local:{tmpdir}"

    # 3. This walrus build allows only one sync-wait command on a CTRL
    # (Drain) instruction; split the tile-exit drain's waits onto nops.
    import concourse.tile as tile
    from concourse import mybir
    from concourse.vector_clock import ScopedClock

    if getattr(tile.TileContext._drain_and_barrier, "_is_patched", False):
        return

    def _patched_drain_and_barrier(self, tick_clock, wait_clock):
        nc = self.nc
        drain_inst = nc.sync.drain()
        wait_clock.add_sem_waits(
            drain_inst.ins, ScopedClock({None: tick_clock.global_clock})
        )
        ow = drain_inst.ins.sync_info.on_wait if drain_inst.ins.sync_info else None
        maxw = 1
        if ow and len(ow) > maxw:
            extra = list(ow[maxw:])
            del ow[maxw:]
            for i in range(0, len(extra), maxw):
                nop = nc.sync.nop(hint="drain_split", nofuse=True)
                if nop.ins.sync_info is None:
                    nop.ins.sync_info = mybir.SyncInfo(on_wait=[], on_update=[])
                for w in extra[i : i + maxw]:
                    nop.ins.sync_info.on_wait.append(w)
        nc.all_engine_barrier()
        assert self.sems is not None
        popped = nc._tile_sem_poison_stack.pop()
        assert popped is self._sem_poison
        nc.clear_and_free_semaphores(list(self.sems.allocated().values()))
        nc.all_engine_barrier()

    _patched_drain_and_barrier._is_patched = True
    tile.TileContext._drain_and_barrier = _patched_drain_and_barrier


def _split_multi_waits(nc):
    """This walrus build allows one sync-wait command per instruction.

    Tile's sem assignment can attach several; move the extras onto nofuse
    NoOps inserted just before the instruction on the same engine (engines
    execute a block's instructions in order, so semantics are unchanged).
    """
    import bass_rust
    from concourse import mybir

    ctr = 0
    for f in nc.m.functions:
        for bb in f.blocks:
            new = []
            changed = False
            for inst in bb.instructions:
                si = inst.sync_info
                ow = si.on_wait if si else None
                if ow is not None and len(ow) > 1:
                    extra = list(ow[:-1])
                    del ow[:-1]
                    for w in extra:
                        ctr += 1
                        nop = bass_rust.InstNoOp()
                        nop.name = f"I-wsplit-{ctr}"
                        nop.engine = inst.engine
                        nop.sync_info = mybir.SyncInfo(on_wait=[w], on_update=[])
                        nop.bass_nofuse = True
                        new.append(nop)
                    changed = True
                new.append(inst)
            if changed:
                bb.instructions = new


def _chunk_sizes(cap):
    """Split cap token columns into chunks of <=512 (PSUM bank limit).

    The first chunk is as large as possible: it runs while the weights are
    still streaming in from HBM, and a wider chunk does more PE work per
    weight byte (lower demand bandwidth during the ramp). The remaining
    columns are split near-equally so no chunk is tiny (per-matmul overhead
    is paid per chunk; a tiny chunk is LDWEIGHTS-bound).
    """
    if cap <= 512:
        return [cap]
    first = 512
    rest = cap - first
    n = max(1, math.ceil(rest / 512))
    base = rest // n
    rem = rest - base * n
    return [first] + [base + (1 if i < rem else 0) for i in range(n)]


_NC_CACHE = {}


def _build_nc(cap):
    if cap in _NC_CACHE:
        return _NC_CACHE[cap]
    import concourse.bass as bass
    import concourse.tile as tile
    from concourse import mybir

    f32 = mybir.dt.float32
    bf16 = mybir.dt.bfloat16
    KD = D // 128  # 8  k-tiles over the model dim
    KH = H // 128  # 16 k-tiles over the hidden dim
    chunks = _chunk_sizes(cap)
    cmax = max(chunks)

    nc = bass.Bass()
    xT = nc.dram_tensor("xT", [D, cap], bf16, kind="ExternalInput")
    wgT = nc.dram_tensor("wgT", [D, H], bf16, kind="ExternalInput")
    wuT = nc.dram_tensor("wuT", [D, H], bf16, kind="ExternalInput")
    wdT = nc.dram_tensor("wdT", [H, D], bf16, kind="ExternalInput")
    out = nc.dram_tensor("out", [D, cap], bf16, kind="ExternalOutput")

    WCH = 512  # weight DMA column-chunk
    c_offs = []
    c0 = 0
    for cn in chunks:
        c_offs.append((c0, cn))
        c0 += cn

    with tile.TileContext(nc) as tc:
        with (
            tc.tile_pool(name="wpool", bufs=1) as wpool,
            tc.tile_pool(name="hpool", bufs=2) as hpool,
            tc.tile_pool(name="opool", bufs=4) as opool,
            tc.tile_pool(name="psum", bufs=2, space="PSUM") as psum,
        ):
            x_sb = [
                wpool.tile([128, cap], bf16, tag=f"x{ki}", name=f"x_sb{ki}")
                for ki in range(KD)
            ]
            wg_sb = [
                wpool.tile([128, H], bf16, tag=f"wg{ki}", name=f"wg_sb{ki}")
                for ki in range(KD)
            ]
            wu_sb = [
                wpool.tile([128, H], bf16, tag=f"wu{ki}", name=f"wu_sb{ki}")
                for ki in range(KD)
            ]
            wd_sb = [
                wpool.tile([128, D], bf16, tag=f"wd{hk}", name=f"wd_sb{hk}")
                for hk in range(KH)
            ]
            cn0 = chunks[0]

            def dma_wg(eng, wc):
                for ki in range(KD):
                    eng.dma_start(
                        wg_sb[ki][:, wc : wc + WCH],
                        wgT[128 * ki : 128 * (ki + 1), wc : wc + WCH],
                    )

            dma_wg(nc.gpsimd, 0)
            dma_wg(nc.scalar, WCH)
            for ki in range(KD):
                nc.sync.dma_start(
                    x_sb[ki][:, :cn0], xT[128 * ki : 128 * (ki + 1), :cn0]
                )
            dma_wg(nc.gpsimd, 2 * WCH)
            dma_wg(nc.gpsimd, 3 * WCH)
            for wc in range(0, H, 2 * WCH):
                for ki in range(KD):
                    nc.sync.dma_start(
                        wu_sb[ki][:, wc : wc + 2 * WCH],
                        wuT[128 * ki : 128 * (ki + 1), wc : wc + 2 * WCH],
                    )
            if cn0 < cap:
                for ki in range(KD):
                    nc.sync.dma_start(
                        x_sb[ki][:, cn0:], xT[128 * ki : 128 * (ki + 1), cn0:]
                    )
            for hk in range(KH):
                nc.gpsimd.dma_start(wd_sb[hk][:], wdT[128 * hk : 128 * (hk + 1), :])

            def gate_up(c0, cn):
                h_sb = hpool.tile([128, KH * cmax], bf16, tag="h", name="h_sb")
                csl = slice(c0, c0 + cn)

                def phase(w_sb, writer):
                    for g0 in range(0, KH, 6):
                        his = range(g0, min(g0 + 6, KH))
                        pp = [
                            psum.tile(
                                [128, 512], f32, tag=f"pp{j}", bufs=1, name=f"pp{j}"
                            )
                            for j in range(len(his))
                        ]
                        for ki in range(KD):
                            for j, hi in enumerate(his):
                                nc.tensor.matmul(
                                    pp[j][:, :cn],
                                    w_sb[ki][:, 128 * hi : 128 * (hi + 1)],
                                    x_sb[ki][:, csl],
                                    start=(ki == 0),
                                    stop=(ki == KD - 1),
                                )
                        for j, hi in enumerate(his):
                            writer(hi, pp[j])

                def gate_writer(hi, pp):
                    nc.scalar.activation(
                        h_sb[:, cmax * hi : cmax * hi + cn],
                        pp[:, :cn],
                        mybir.ActivationFunctionType.Silu,
                    )

                def up_writer(hi, pp):
                    hslc = slice(cmax * hi, cmax * hi + cn)
                    nc.vector.tensor_mul(h_sb[:, hslc], h_sb[:, hslc], pp[:, :cn])

                phase(wg_sb, gate_writer)
                phase(wu_sb, up_writer)
                return h_sb

            def down(h_sb, c0, cn):
                for di in range(KD):
                    dsl = slice(128 * di, 128 * (di + 1))
                    po = psum.tile([128, 512], f32, tag="po", name="po")
                    for hk in range(KH):
                        nc.tensor.matmul(
                            po[:, :cn],
                            wd_sb[hk][:, dsl],
                            h_sb[:, cmax * hk : cmax * hk + cn],
                            start=(hk == 0),
                            stop=(hk == KH - 1),
                        )
                    o = opool.tile([128, 512], bf16, tag="o", name="o")
                    nc.vector.tensor_copy(o[:, :cn], po[:, :cn])
                    nsp = 4 if (c0, cn) == c_offs[-1] else 2
                    cb = [cn * i // nsp for i in range(nsp + 1)]
                    for i in range(nsp):
                        nc.sync.dma_start(
                            out[dsl, c0 + cb[i] : c0 + cb[i + 1]],
                            o[:, cb[i] : cb[i + 1]],
                        )

            prev = None
            for c0i, cni in c_offs:
                h_sb = gate_up(c0i, cni)
                if prev is not None:
                    down(*prev)
                prev = (h_sb, c0i, cni)
            down(*prev)
    _split_multi_waits(nc)
    _NC_CACHE[cap] = nc
    return nc


def kernel(x, expert_indices, w_gate, w_up, w_down):
    global LAST_RESULT
    _install_shims()
    from concourse import bass_utils

    x = np.asarray(x)
    ei = np.asarray(expert_indices).astype(np.int64)
    w_gate = np.asarray(w_gate)
    w_up = np.asarray(w_up)
    w_down = np.asarray(w_down)

    flat = ei.reshape(-1)  # pair p = t*A + a  ->  expert id
    # Dedup: a (token, slot) pair whose expert already appears in an earlier
    # slot of the same token produces an identical output row — compute the
    # first occurrence only and copy the result to the duplicates afterward.
    keep = np.ones(T * A, dtype=bool)
    for a in range(1, A):
        dup_any = np.zeros(T, dtype=bool)
        for b in range(a):
            dup_any |= ei[:, a] == ei[:, b]
        keep[a::A] = ~dup_any[: T]
    kept = np.nonzero(keep)[0]
    flat_kept = flat[kept]
    counts = np.bincount(flat_kept, minlength=E)
    order = np.argsort(flat_kept, kind="stable")
    starts = np.zeros(E + 1, dtype=np.int64)
    np.cumsum(counts, out=starts[1:])
    cap = int(counts.max())
    cap = max(cap, 128)

    idx_per_core = []
    in_maps = []
    for e in range(E):
        idx = kept[order[starts[e] : starts[e + 1]]]  # original pair ids
        idx_per_core.append(idx)
        tok = idx // A
        xeT = np.zeros((D, cap), dtype=BF16)
        xeT[:, : len(idx)] = x[tok].T.astype(BF16)
        in_maps.append(
            {
                "xT": xeT,
                "wgT": np.ascontiguousarray(w_gate[e].T).astype(BF16),
                "wuT": np.ascontiguousarray(w_up[e].T).astype(BF16),
                "wdT": np.ascontiguousarray(w_down[e].T).astype(BF16),
            }
        )

    nc = _build_nc(cap)
    res = bass_utils.run_bass_kernel_spmd(nc, in_maps, core_ids=list(range(N_CORES)))
    LAST_RESULT = res

    out = np.zeros((T * A, D), dtype=np.float32)
    for e in range(E):
        idx = idx_per_core[e]
        oT = np.asarray(res.results[e]["out"])  # [D, cap] bf16
        out[idx] = oT[:, : len(idx)].T.astype(np.float32)
    out = out.reshape(T, A, D)
    for a in range(1, A):  # fill duplicate slots from their first occurrence
        for b in range(a):
            m = ei[:, a] == ei[:, b]
            if b > 0:
                for c in range(b):
                    m &= ei[:, b] != ei[:, c]  # b is itself the first occurrence
            out[m, a] = out[m, b]
    return out



# revision 17
# speedup vs baseline: 1.0257x; 1.0076x over previous
"""Expert-parallel MoE feed-forward (top-2 routing) on 8 TRN2 NeuronCores.

Strategy: one expert per core (E == n_cores == 8). Token routing is part of
input sharding: host gathers each expert's assigned token activations
(transposed, bf16) and feeds core e only its tokens plus its expert's three
weight matrices. Each core runs a dense FFN
    out = (silu(x @ Wg^T) * (x @ Wu^T)) @ Wd^T
over its token batch in bf16 (fp32 PSUM accumulation), entirely from SBUF.
Host scatters per-core outputs back into the (T, A, D) result.
"""

import math
import sys
import types

import numpy as np
import ml_dtypes

T, D, H, E, A = 4096, 1024, 2048, 8, 2
N_CORES = 8
BF16 = ml_dtypes.bfloat16

# Filled by kernel() with the BassKernelResults of the last device run so an
# external harness (test.py) can read exec_time_ns when tracing is on.
LAST_RESULT = None

_SHIMS_DONE = False


def _install_shims():
    """Environment fixes for running Bass/Tile SPMD kernels under axon."""
    global _SHIMS_DONE
    if _SHIMS_DONE:
        return
    _SHIMS_DONE = True

    # 1. NTFF profile hook (lets trace=True / BASS_TRACE=1 report exec_time_ns).
    if "antenv.axon_hooks" not in sys.modules:
        try:
            import antenv.axon_hooks  # noqa: F401  (real module present)
        except ImportError:
            _hook = None
            try:
                import trn_agent_boot.trn_boot as tb

                _hook = tb._ntff_profile_via_ctypes("/opt/axon/libaxon_pjrt.so")
            except Exception:
                _hook = None
            mod = types.ModuleType("antenv.axon_hooks")
            mod.get_axon_ntff_profile_hook = lambda: _hook
            sys.modules["antenv.axon_hooks"] = mod

    # 2. No artifact upload from a zero-egress container.
    from concourse import bass_utils

    bass_utils.upload_artifacts = lambda tmpdir: f"local:{tmpdir}"

    # 3. This walrus build allows only one sync-wait command on a CTRL
    # (Drain) instruction; split the tile-exit drain's waits onto nops.
    import concourse.tile as tile
    from concourse import mybir
    from concourse.vector_clock import ScopedClock

    if getattr(tile.TileContext._drain_and_barrier, "_is_patched", False):
        return

    def _patched_drain_and_barrier(self, tick_clock, wait_clock):
        nc = self.nc
        drain_inst = nc.sync.drain()
        wait_clock.add_sem_waits(
            drain_inst.ins, ScopedClock({None: tick_clock.global_clock})
        )
        ow = drain_inst.ins.sync_info.on_wait if drain_inst.ins.sync_info else None
        maxw = 1
        if ow and len(ow) > maxw:
            extra = list(ow[maxw:])
            del ow[maxw:]
            for i in range(0, len(extra), maxw):
                nop = nc.sync.nop(hint="drain_split", nofuse=True)
                if nop.ins.sync_info is None:
                    nop.ins.sync_info = mybir.SyncInfo(on_wait=[], on_update=[])
                for w in extra[i : i + maxw]:
                    nop.ins.sync_info.on_wait.append(w)
        nc.all_engine_barrier()
        assert self.sems is not None
        popped = nc._tile_sem_poison_stack.pop()
        assert popped is self._sem_poison
        nc.clear_and_free_semaphores(list(self.sems.allocated().values()))
        nc.all_engine_barrier()

    _patched_drain_and_barrier._is_patched = True
    tile.TileContext._drain_and_barrier = _patched_drain_and_barrier


def _split_multi_waits(nc):
    """This walrus build allows one sync-wait command per instruction.

    Tile's sem assignment can attach several; move the extras onto nofuse
    NoOps inserted just before the instruction on the same engine (engines
    execute a block's instructions in order, so semantics are unchanged).
    """
    import bass_rust
    from concourse import mybir

    ctr = 0
    for f in nc.m.functions:
        for bb in f.blocks:
            new = []
            changed = False
            for inst in bb.instructions:
                si = inst.sync_info
                ow = si.on_wait if si else None
                if ow is not None and len(ow) > 1:
                    extra = list(ow[:-1])
                    del ow[:-1]
                    for w in extra:
                        ctr += 1
                        nop = bass_rust.InstNoOp()
                        nop.name = f"I-wsplit-{ctr}"
                        nop.engine = inst.engine
                        nop.sync_info = mybir.SyncInfo(on_wait=[w], on_update=[])
                        nop.bass_nofuse = True
                        new.append(nop)
                    changed = True
                new.append(inst)
            if changed:
                bb.instructions = new


def _chunk_sizes(cap):
    """Split cap token columns into chunks of <=512 (PSUM bank limit).

    The first chunk is as large as possible: it runs while the weights are
    still streaming in from HBM, and a wider chunk does more PE work per
    weight byte (lower demand bandwidth during the ramp). The remaining
    columns are split near-equally so no chunk is tiny (per-matmul overhead
    is paid per chunk; a tiny chunk is LDWEIGHTS-bound).
    """
    if cap <= 512:
        return [cap]
    first = 512
    rest = cap - first
    n = max(1, math.ceil(rest / 512))
    base = rest // n
    rem = rest - base * n
    return [first] + [base + (1 if i < rem else 0) for i in range(n)]


_NC_CACHE = {}


def _build_nc(cap):
    if cap in _NC_CACHE:
        return _NC_CACHE[cap]
    import concourse.bass as bass
    import concourse.tile as tile
    from concourse import mybir

    f32 = mybir.dt.float32
    bf16 = mybir.dt.bfloat16
    KD = D // 128  # 8  k-tiles over the model dim
    KH = H // 128  # 16 k-tiles over the hidden dim
    chunks = _chunk_sizes(cap)
    cmax = max(chunks)

    nc = bass.Bass()
    xT = nc.dram_tensor("xT", [D, cap], bf16, kind="ExternalInput")
    wgT = nc.dram_tensor("wgT", [D, H], bf16, kind="ExternalInput")
    wuT = nc.dram_tensor("wuT", [D, H], bf16, kind="ExternalInput")
    wdT = nc.dram_tensor("wdT", [H, D], bf16, kind="ExternalInput")
    out = nc.dram_tensor("out", [D, cap], bf16, kind="ExternalOutput")

    WCH = 512  # weight DMA column-chunk
    c_offs = []
    c0 = 0
    for cn in chunks:
        c_offs.append((c0, cn))
        c0 += cn

    with tile.TileContext(nc) as tc:
        with (
            tc.tile_pool(name="wpool", bufs=1) as wpool,
            tc.tile_pool(name="hpool", bufs=2) as hpool,
            tc.tile_pool(name="opool", bufs=4) as opool,
            tc.tile_pool(name="psum", bufs=2, space="PSUM") as psum,
        ):
            x_sb = [
                wpool.tile([128, cap], bf16, tag=f"x{ki}", name=f"x_sb{ki}")
                for ki in range(KD)
            ]
            wg_sb = [
                wpool.tile([128, H], bf16, tag=f"wg{ki}", name=f"wg_sb{ki}")
                for ki in range(KD)
            ]
            wu_sb = [
                wpool.tile([128, H], bf16, tag=f"wu{ki}", name=f"wu_sb{ki}")
                for ki in range(KD)
            ]
            wd_sb = [
                wpool.tile([128, D], bf16, tag=f"wd{hk}", name=f"wd_sb{hk}")
                for hk in range(KH)
            ]
            cn0 = chunks[0]

            # The PE clock-gate (HAM) needs ~3.4 us of sustained activity to
            # release the 1.2 GHz cold throttle, and real matmuls can't start
            # until the first DMAs land. Burn the wait on a short burst of
            # dummy matmuls so the busy window starts early.
            scr = wpool.tile([128, 128], bf16, tag="scr", name="scr")
            nc.any.memset(scr[:], 0.0)
            wm = psum.tile([128, 512], f32, tag="po", name="po")
            for _ in range(8):
                nc.tensor.matmul(
                    wm[:, :64], scr[:, :], scr[:, :64], start=True, stop=True
                )

            def dma_wg(eng, wc, k_from=0):
                for ki in range(k_from, KD):
                    eng.dma_start(
                        wg_sb[ki][:, wc : wc + WCH],
                        wgT[128 * ki : 128 * (ki + 1), wc : wc + WCH],
                    )

            # First DMA on ACT is wg[k0] chunk0 and on SP is x[k0] chunk0 —
            # exactly what the first matmul needs. _hoist_first_dmas moves
            # these two above the tile-entry barrier so their transfers run
            # during the framework preamble instead of after it.
            nc.scalar.dma_start(wg_sb[0][:, :WCH], wgT[0:128, :WCH])
            dma_wg(nc.gpsimd, 0, k_from=1)
            dma_wg(nc.scalar, WCH)
            for ki in range(KD):
                nc.sync.dma_start(
                    x_sb[ki][:, :cn0], xT[128 * ki : 128 * (ki + 1), :cn0]
                )
            dma_wg(nc.gpsimd, 2 * WCH)
            dma_wg(nc.gpsimd, 3 * WCH)
            for wc in range(0, H, 2 * WCH):
                for ki in range(KD):
                    nc.sync.dma_start(
                        wu_sb[ki][:, wc : wc + 2 * WCH],
                        wuT[128 * ki : 128 * (ki + 1), wc : wc + 2 * WCH],
                    )
            if cn0 < cap:
                for ki in range(KD):
                    nc.sync.dma_start(
                        x_sb[ki][:, cn0:], xT[128 * ki : 128 * (ki + 1), cn0:]
                    )
            for hk in range(KH):
                nc.gpsimd.dma_start(wd_sb[hk][:], wdT[128 * hk : 128 * (hk + 1), :])

            def gate_up(c0, cn):
                h_sb = hpool.tile([128, KH * cmax], bf16, tag="h", name="h_sb")
                csl = slice(c0, c0 + cn)

                def phase(w_sb, writer):
                    for g0 in range(0, KH, 6):
                        his = range(g0, min(g0 + 6, KH))
                        pp = [
                            psum.tile(
                                [128, 512], f32, tag=f"pp{j}", bufs=1, name=f"pp{j}"
                            )
                            for j in range(len(his))
                        ]
                        for ki in range(KD):
                            for j, hi in enumerate(his):
                                nc.tensor.matmul(
                                    pp[j][:, :cn],
                                    w_sb[ki][:, 128 * hi : 128 * (hi + 1)],
                                    x_sb[ki][:, csl],
                                    start=(ki == 0),
                                    stop=(ki == KD - 1),
                                )
                        for j, hi in enumerate(his):
                            writer(hi, pp[j])

                def gate_writer(hi, pp):
                    nc.scalar.activation(
                        h_sb[:, cmax * hi : cmax * hi + cn],
                        pp[:, :cn],
                        mybir.ActivationFunctionType.Silu,
                    )

                def up_writer(hi, pp):
                    hslc = slice(cmax * hi, cmax * hi + cn)
                    nc.vector.tensor_mul(h_sb[:, hslc], h_sb[:, hslc], pp[:, :cn])

                phase(wg_sb, gate_writer)
                phase(wu_sb, up_writer)
                return h_sb

            def down(h_sb, c0, cn):
                # Rotate PSUM banks through all 8 (the 6 gate/up banks are
                # idle by now): with only 2 "po" banks each d-tile's first
                # matmul hits a WAR stall waiting for the previous d-tile's
                # PSUM->SBUF copy; an 8-deep rotation gives the copy several
                # d-tile spans to finish.
                last_chunk = (c0, cn) == c_offs[-1]
                for di in range(KD):
                    dsl = slice(128 * di, 128 * (di + 1))
                    if di < 6:
                        po = psum.tile(
                            [128, 512], f32, tag=f"pp{di}", bufs=1, name=f"pp{di}"
                        )
                    else:
                        po = psum.tile([128, 512], f32, tag="po", name="po")
                    if last_chunk and di == KD - 1:
                        # The exec-time clock stops at output completion, so
                        # the last tile's drain is fully exposed: accumulate
                        # it in two column halves (separate PSUM banks — the
                        # tile tracker is coarse) so half A's copy and store
                        # run under half B's matmuls.
                        h1 = cn // 2
                        po_b = psum.tile([128, 512], f32, tag="po", name="po")
                        o = opool.tile([128, 512], bf16, tag="o", name="o")
                        for lo, hi, pb, cpv in (
                            (0, h1, po, True),
                            (h1, cn, po_b, False),
                        ):
                            for hk in range(KH):
                                nc.tensor.matmul(
                                    pb[:, lo:hi],
                                    wd_sb[hk][:, dsl],
                                    h_sb[:, cmax * hk + lo : cmax * hk + hi],
                                    start=(hk == 0),
                                    stop=(hk == KH - 1),
                                )
                            if cpv:
                                nc.vector.tensor_copy(o[:, lo:hi], pb[:, lo:hi])
                            else:
                                nc.scalar.activation(
                                    o[:, lo:hi],
                                    pb[:, lo:hi],
                                    mybir.ActivationFunctionType.Copy,
                                )
                            nc.sync.dma_start(
                                out[dsl, c0 + lo : c0 + hi], o[:, lo:hi]
                            )
                    else:
                        for hk in range(KH):
                            nc.tensor.matmul(
                                po[:, :cn],
                                wd_sb[hk][:, dsl],
                                h_sb[:, cmax * hk : cmax * hk + cn],
                                start=(hk == 0),
                                stop=(hk == KH - 1),
                            )
                        o = opool.tile([128, 512], bf16, tag="o", name="o")
                        nc.vector.tensor_copy(o[:, :cn], po[:, :cn])
                        nsp = 2
                        cb = [cn * i // nsp for i in range(nsp + 1)]
                        for i in range(nsp):
                            nc.sync.dma_start(
                                out[dsl, c0 + cb[i] : c0 + cb[i + 1]],
                                o[:, cb[i] : cb[i + 1]],
                            )

            prev = None
            for c0i, cni in c_offs:
                h_sb = gate_up(c0i, cni)
                if prev is not None:
                    down(*prev)
                prev = (h_sb, c0i, cni)
            down(*prev)
    _hoist_first_dmas(nc)
    _split_multi_waits(nc)
    _NC_CACHE[cap] = nc
    return nc


def _hoist_first_dmas(nc):
    """Start the first x / wg transfers during the framework preamble.

    The measured exec window opens at NEFF start, but input DMAs normally
    only issue after ~1.3 us of tile-entry barrier + branch. Move the first
    DMACopy on SP (x[k0] chunk0) and ACT (wg[k0] chunk0) into block 0, ahead
    of that engine's entry-barrier Drain, and replace the Drain with a NoOp
    carrying the same sem waits/updates — the Drain polls the engine's DMA
    queues and would otherwise block the barrier on the in-flight transfer.
    Safety: the hoisted DMAs' completion increments land ~0.5 us after
    gpsimd's init-time semaphore range-clear, and their issue order within
    the engine is unchanged, so all tile-assigned thresholds still hold.
    """
    import bass_rust
    from concourse import mybir

    f = nc.m.functions[0]
    if len(f.blocks) < 2:
        return
    b0, b1 = f.blocks[0], f.blocks[1]
    for eng in (mybir.EngineType.SP, mybir.EngineType.Activation):
        dma = None
        rest = []
        for inst in b1.instructions:
            if dma is None and inst.engine == eng and type(inst).__name__ == "InstDMACopy":
                dma = inst
                continue
            rest.append(inst)
        if dma is None:
            continue
        new0 = []
        placed = False
        for inst in b0.instructions:
            if (
                not placed
                and inst.engine == eng
                and type(inst).__name__ == "InstDrain"
            ):
                new0.append(dma)
                nop = bass_rust.InstNoOp()
                nop.name = f"I-hoistdrain-{eng}"
                nop.engine = inst.engine
                nop.sync_info = inst.sync_info
                nop.bass_nofuse = True
                new0.append(nop)
                placed = True
                continue
            new0.append(inst)
        if placed:
            b0.instructions = new0
            b1.instructions = rest


def kernel(x, expert_indices, w_gate, w_up, w_down):
    global LAST_RESULT
    _install_shims()
    from concourse import bass_utils

    x = np.asarray(x)
    ei = np.asarray(expert_indices).astype(np.int64)
    w_gate = np.asarray(w_gate)
    w_up = np.asarray(w_up)
    w_down = np.asarray(w_down)

    flat = ei.reshape(-1)  # pair p = t*A + a  ->  expert id
    # Dedup: a (token, slot) pair whose expert already appears in an earlier
    # slot of the same token produces an identical output row — compute the
    # first occurrence only and copy the result to the duplicates afterward.
    keep = np.ones(T * A, dtype=bool)
    for a in range(1, A):
        dup_any = np.zeros(T, dtype=bool)
        for b in range(a):
            dup_any |= ei[:, a] == ei[:, b]
        keep[a::A] = ~dup_any[: T]
    kept = np.nonzero(keep)[0]
    flat_kept = flat[kept]
    counts = np.bincount(flat_kept, minlength=E)
    order = np.argsort(flat_kept, kind="stable")
    starts = np.zeros(E + 1, dtype=np.int64)
    np.cumsum(counts, out=starts[1:])
    cap = int(counts.max())
    cap = max(cap, 128)

    idx_per_core = []
    in_maps = []
    for e in range(E):
        idx = kept[order[starts[e] : starts[e + 1]]]  # original pair ids
        idx_per_core.append(idx)
        tok = idx // A
        xeT = np.zeros((D, cap), dtype=BF16)
        xeT[:, : len(idx)] = x[tok].T.astype(BF16)
        in_maps.append(
            {
                "xT": xeT,
                "wgT": np.ascontiguousarray(w_gate[e].T).astype(BF16),
                "wuT": np.ascontiguousarray(w_up[e].T).astype(BF16),
                "wdT": np.ascontiguousarray(w_down[e].T).astype(BF16),
            }
        )

    nc = _build_nc(cap)
    res = bass_utils.run_bass_kernel_spmd(nc, in_maps, core_ids=list(range(N_CORES)))
    LAST_RESULT = res

    out = np.zeros((T * A, D), dtype=np.float32)
    for e in range(E):
        idx = idx_per_core[e]
        oT = np.asarray(res.results[e]["out"])  # [D, cap] bf16
        out[idx] = oT[:, : len(idx)].T.astype(np.float32)
    out = out.reshape(T, A, D)
    for a in range(1, A):  # fill duplicate slots from their first occurrence
        for b in range(a):
            m = ei[:, a] == ei[:, b]
            if b > 0:
                for c in range(b):
                    m &= ei[:, b] != ei[:, c]  # b is itself the first occurrence
            out[m, a] = out[m, b]
    return out

